# revision 1
# baseline (speedup 1.0000x reference)
"""MoE (8 experts, top-2, shared expert) Trainium2 kernel.

Strategy (expert-parallel, per sharding hint):
  - Host computes routing (sigmoid gate -> top-2 -> stable sort by expert),
    exactly mirroring the jax reference in fp32 numpy, and plays the role of
    the all-to-all: expert e's tokens (scaled by router scores, bf16,
    zero-padded to capacity C) go to core e. The shared expert is
    data-parallel: core i gets tokens [i*256, (i+1)*256).
  - Device does the 4 GEMMs in feature-major layout (tokens on the moving
    free dim) so no on-chip transposes are needed:
        hT  = wu.T.T @ xrT  (bf16, fp32 accum)   -> relu^2 in bf16
        yrT = wd.T.T @ hT   (bf16, fp32 accum)   -> bf16 out
        gT  = su.T.T @ xsT  (fp16)               -> relu^2
        ysT = sd.T.T @ gT   (fp16)               -> fp32 out
    fp16 (10-bit mantissa) gives ~tf32 precision at half the f32 DMA bytes.
  - Host scatters per-expert outputs back to token order, sums top-2 + shared.

Hardware quirks baked into the structure (trn2 walrus codegen):
  - Inputs arrive as a few packed, k-split DMA streams (wx = [w_up.T|xr.T],
    ssd = [shared up weights + xs | shared down weights]); epilogues run on
    DVE (ACT is several times slower on plain relu/copy); Bacc.compile()
    legalizes sync-wait budgets (matmul waits move onto ldweights, excess
    waits split into event semaphores).

Self-contained: hardcodes shapes from the problem spec.
"""

import numpy as np
import ml_dtypes
from contextlib import ExitStack

T = 2048          # tokens (BS*SLEN)
DIM = 1024
E = 8             # experts == cores
TOPK = 2
HID = 1408
NCORES = 8
S = T // NCORES   # shared-expert tokens per core

KD = DIM // 128   # 8  k-tiles contracting over DIM
MH = HID // 128   # 11 m-tiles over hidden
MD = DIM // 128   # 8  m-tiles over model dim

TRACE = False
TRACE_CORES = None
TRACE_DIR = None
LAST_RESULT = None   # BassKernelResults of the last run (for test harness)

_PROG_CACHE = {}

bf16 = ml_dtypes.bfloat16


def _chunks(total, step=512):
    out = []
    o = 0
    while o < total:
        c = min(step, total - o)
        out.append((o, c))
        o += c
    return out


def _build_program(C):
    import concourse.tile as tile
    import concourse.mybir as mybir
    from concourse import bacc

    dt = mybir.dt
    # Bacc (not raw Bass): its compile() pass moves matmul waits onto
    # ldweights and splits over-budget sync waits into event semaphores —
    # without it walrus rejects instructions with >1 wait.
    nc = bacc.Bacc("TRN2", target_bir_lowering=False)

    WXW = HID + C         # per-k width of [w_up.T | xr.T] pack
    SSW = HID + S         # per-k width of [shared_w_up.T | xs.T] pack
    SDO = KD * SSW        # offset of the sd pack inside ssd
    SSDW = SDO + MH * DIM

    YW = C + 2 * S     # bf16 out width: [yr bf16 | ys f32 as 2x bf16 slots]

    wxT = nc.declare_dram_parameter("wxT", [128, KD, WXW], dt.bfloat16,
                                    isOutput=False)
    wdT = nc.declare_dram_parameter("wdT", [128, MH, DIM], dt.bfloat16,
                                    isOutput=False)
    ssdT = nc.declare_dram_parameter("ssdT", [128, SSDW], dt.float16,
                                     isOutput=False)
    # Single output param (one store DMA): the drain's wait list covers
    # PE + DVE + every used DMA queue and holds at most 6 entries.
    yT = nc.declare_dram_parameter("yT", [128, MD, YW], dt.bfloat16,
                                   isOutput=True)

    CCH = _chunks(C)   # routed token chunks (<=512)
    SCH = _chunks(S)   # shared token chunks

    with ExitStack() as ctx:
        tc = ctx.enter_context(tile.TileContext(nc))
        wpool = ctx.enter_context(tc.tile_pool(name="w", bufs=1))
        hpool = ctx.enter_context(tc.tile_pool(name="h", bufs=1))
        opool = ctx.enter_context(tc.tile_pool(name="o", bufs=1))
        psA = ctx.enter_context(tc.tile_pool(name="psA", bufs=4, space="PSUM"))
        psB = ctx.enter_context(tc.tile_pool(name="psB", bufs=2, space="PSUM"))
        psS = ctx.enter_context(tc.tile_pool(name="psS", bufs=2, space="PSUM"))

        # Loads split along k so the PE can start as soon as the first chunk
        # lands (Bacc's compile pass legalizes any over-budget sync waits).
        from concourse.tile_rust import add_dep_helper
        wx = wpool.tile([128, KD, WXW], dt.bfloat16, tag="wx", name="wx")
        wx_last = None
        for k0, k1 in ((0, 1), (1, 2), (2, 4), (4, 6), (6, 8)):
            wx_last = nc.sync.dma_start(wx[:, k0:k1, :], wxT[:, k0:k1, :])
        ssd = wpool.tile([128, SSDW], dt.float16, tag="ssd", name="ssd")
        late = []
        for i, (o0, o1) in enumerate(((0, SDO // 2), (SDO // 2, SDO),
                                      (SDO, SDO + 6 * DIM),
                                      (SDO + 6 * DIM, SSDW))):
            di = nc.sync.dma_start(ssd[:, o0:o1], ssdT[:, o0:o1])
            if i > 1:
                late.append(di)
        wd = wpool.tile([128, MH, DIM], dt.bfloat16, tag="wd", name="wd")
        for k0, k1 in ((0, 6), (6, MH)):
            late.append(nc.sync.dma_start(wd[:, k0:k1, :], wdT[:, k0:k1, :]))
        # Loads consumed mid-kernel wait for wx so the startup-critical wx
        # stream gets the full DMA bandwidth.
        for di in late:
            add_dep_helper(di.ins, wx_last.ins,
                           reason="late loads yield DMA bandwidth to wx")

        def wu_w(k, ms):
            return wx[:, k, ms]

        def xr_r(k, co, cw):
            return wx[:, k, HID + co:HID + co + cw]

        def su_w(k, ms):
            return ssd[:, k * SSW + ms.start:k * SSW + ms.stop]

        def xs_r(k, co, cw):
            return ssd[:, k * SSW + HID + co:k * SSW + HID + co + cw]

        def sd_w(k, ms):
            return ssd[:, SDO + k * DIM + ms.start:SDO + k * DIM + ms.stop]

        def wd_w(k, ms):
            return wd[:, k, ms]

        def mm_phase(m_tiles, k_tiles, w_fn, r_fn, chunks, shared):
            """One GEMM phase; yields (m, [(co, cw, psum)]) after accumulation."""
            for m in range(m_tiles):
                ms = slice(m * 128, (m + 1) * 128)
                pss = []
                for (co, cw) in chunks:
                    pool = psS if shared else (psA if cw > 128 else psB)
                    tg = pool.name
                    ps = pool.tile([128, cw], dt.float32, tag=tg, name=tg)
                    pss.append((co, cw, ps))
                for k in range(k_tiles):
                    for (co, cw, ps) in pss:
                        nc.tensor.matmul(ps[:], w_fn(k, ms), r_fn(k, co, cw),
                                         start=(k == 0), stop=(k == k_tiles - 1))
                yield m, pss

        # Epilogues all on DVE (ACT pays a LUT-table load per op and is
        # several times slower on plain relu/copy tiles).
        # --- routed + shared up-projs, m-tiles interleaved so the PE has
        # shared work to run while early wx chunks are still in flight ---
        h_t = hpool.tile([128, MH, C], dt.bfloat16, tag="h", name="h")
        g_t = hpool.tile([128, MH, S], dt.float16, tag="g", name="g")
        g1r = mm_phase(MH, KD, wu_w, xr_r, CCH, False)
        g1s = mm_phase(MH, KD, su_w, xs_r, SCH, True)
        pattern = ['r', 'r'] + ['r', 's'] * 9 + ['s'] * 2
        for which in pattern:
            item = next(g1r if which == 'r' else g1s, None)
            if item is None:
                continue
            m, pss = item
            dst = h_t if which == 'r' else g_t
            for (co, cw, ps) in pss:
                v = dst[:, m, co:co + cw]
                nc.vector.tensor_relu(v, ps[:])
                nc.vector.tensor_mul(v, v, v)

        def h_r(k, co, cw):
            return h_t[:, k, co:co + cw]

        def g_r(k, co, cw):
            return g_t[:, k, co:co + cw]

        # --- down-projs -> staged stores (overlap the store DMAs with
        # the remaining compute; 4 half-size stores instead of one) ---
        ybr = opool.tile([128, MD, C], dt.bfloat16, tag="ybr", name="ybr")
        for m, pss in mm_phase(MD, MH, wd_w, h_r, CCH, False):
            for (co, cw, ps) in pss:
                nc.vector.tensor_copy(ybr[:, m, co:co + cw], ps[:])
            if m in (MD // 2 - 1, MD - 1):
                m0 = 0 if m < MD // 2 else MD // 2
                nc.sync.dma_start(yT[:, m0:m + 1, :C], ybr[:, m0:m + 1, :])
        ybs = opool.tile([128, MD, S], dt.float32, tag="ybs", name="ybs")
        for m, pss in mm_phase(MD, MH, sd_w, g_r, SCH, True):
            for (co, cw, ps) in pss:
                nc.vector.tensor_copy(ybs[:, m, co:co + cw], ps[:])
            if m in (3, 5, 6, MD - 1):
                m0 = {3: 0, 5: 4, 6: 6, MD - 1: 7}[m]
                nc.sync.dma_start(
                    yT[:, m0:m + 1, C:].bitcast(dt.float32),
                    ybs[:, m0:m + 1, :])

    nc.compile()
    return nc


def _route(x, gate_w, expert_bias):
    """Exact numpy mirror of the reference TopKRouter + dispatch."""
    xf = x.reshape(-1, DIM).astype(np.float32)
    logits = xf @ gate_w.T.astype(np.float32)
    scores = 1.0 / (1.0 + np.exp(-logits.astype(np.float32)))
    biased = scores + expert_bias[None, :].astype(np.float32)
    # top-2, ties -> lower index (matches jax.lax.top_k)
    sel = np.argsort(-biased, axis=-1, kind="stable")[:, :TOPK]
    top_scores = np.take_along_axis(scores, sel, axis=-1)
    flat_sel = sel.reshape(-1)
    counts = np.bincount(flat_sel, minlength=E)
    order = np.argsort(flat_sel, kind="stable")
    scores_sorted = top_scores.reshape(-1)[order]
    token_ids = order // TOPK
    return xf, counts, order, token_ids, scores_sorted


def _kchunk(mat, width):
    """(n_k*128, width) row-major -> (128, n_k, width)."""
    return mat.reshape(-1, 128, width).transpose(1, 0, 2)


def kernel(x, gate_w, expert_bias, w_up, w_down, shared_w_up, shared_w_down):
    global LAST_RESULT
    from concourse.bass_utils import run_bass_kernel_spmd

    xf, counts, order, token_ids, scores_sorted = _route(x, gate_w, expert_bias)

    C = max(576, int(-(-counts.max() // 64) * 64))  # capacity per expert
    starts = np.zeros(E + 1, np.int64)
    np.cumsum(counts, out=starts[1:])

    # dispatch: routed_in rows grouped by expert, scaled by router score
    routed_in = (xf[token_ids] * scores_sorted[:, None]).astype(np.float32)
    routed_in = routed_in.astype(bf16)

    suT = shared_w_up.T.astype(np.float16)      # (DIM, HID)
    sdT = shared_w_down.T.astype(np.float16)    # (HID, DIM)
    sd_pack = _kchunk(sdT, DIM).reshape(128, -1)
    in_maps = []
    for e in range(NCORES):
        seg = routed_in[starts[e]:starts[e + 1]]
        xr = np.zeros((C, DIM), bf16)
        xr[:seg.shape[0]] = seg
        xsT = xf[e * S:(e + 1) * S].T.astype(np.float16)   # (DIM, S)
        wuT = w_up[e].astype(bf16).T                       # (DIM, HID)
        ss = np.concatenate(
            [_kchunk(suT, HID), _kchunk(xsT, S)], axis=2).reshape(128, -1)
        in_maps.append({
            "wxT": np.ascontiguousarray(np.concatenate(
                [_kchunk(wuT, HID), _kchunk(xr.T, C)], axis=2)),
            "wdT": np.ascontiguousarray(_kchunk(w_down[e].astype(bf16).T, DIM)),
            "ssdT": np.ascontiguousarray(np.concatenate([ss, sd_pack], axis=1)),
        })

    if C not in _PROG_CACHE:
        _PROG_CACHE[C] = _build_program(C)
    nc = _PROG_CACHE[C]

    res = run_bass_kernel_spmd(
        nc, in_maps, list(range(NCORES)),
        trace=TRACE,
        trace_cores=TRACE_CORES,
        tmpdir=TRACE_DIR,
    )
    LAST_RESULT = res

    # --- combine (host): scatter per-expert outputs back to token order ---
    routed_sorted = np.empty((T * TOPK, DIM), np.float32)
    for e in range(NCORES):
        arr = np.asarray(res.results[e]["yT"])             # (128, MD, C+2S) bf16
        yr = arr[:, :, :C].transpose(1, 0, 2).reshape(DIM, C).T
        routed_sorted[starts[e]:starts[e + 1]] = yr[:counts[e]].astype(np.float32)
    combined = np.empty((T * TOPK, DIM), np.float32)
    combined[order] = routed_sorted
    out = combined.reshape(T, TOPK, DIM).sum(axis=1)

    for e in range(NCORES):
        arr = np.asarray(res.results[e]["yT"])
        ys = np.ascontiguousarray(arr[:, :, C:]).view(np.float32)  # (128,MD,S)
        out[e * S:(e + 1) * S] += ys.transpose(1, 0, 2).reshape(DIM, S).T

    return out.reshape(1, T, DIM).astype(np.float32)

